# revision 1
# baseline (speedup 1.0000x reference)
"""Trainium2 Bass kernel for nn_BlockUngrouper.

Problem: out[b, n, :] = block_features[b, g, k, :] where g is the block whose
one-hot claims token n and k is n's rank within that block (cumsum of the
one-hot along n).  The input distribution (per-sample permutation partition)
guarantees each token is claimed by exactly one block and ranks < 128, so per
batch this is a row-permutation gather with
    flat_idx[n] = 128 * g(n) + rank(n).

Sharding: data-parallel over the batch dim, 2 batches per NeuronCore x 8.

Per-core program (all index arithmetic exact in fp32/bf16):
  1. onehot [N, 128] -> SBUF bf16, layout [token-in-tile, (tile, g)]
     (dtype cast during SWDGE DMA).
  2. counts[g, t]: per 128-token tile, PE matmul lhsT=OH rhs=ones.
  3. prefix[g, t]: exclusive scan over tiles (DVE tensor_tensor_scan),
     PE-transpose to [t, g], add 128*g - 1.
  4. per 4-tile PSUM group: flatten the 4 prefix rows to partition 0 (tiny
     SBUF->SBUF DMA), broadcast-add via K=1 matmul (start), then 4 upper-
     triangular-matrix matmuls add the within-tile inclusive cumsum.
  5. flat_idx: DVE scalar_tensor_tensor (PSUM x onehot) with accum_out
     reducing over g (the one-hot selects its block's entry).
  6. per tile: one SWDGE indirect DMA gathers 128 x 1KB feature rows
     (hardware contract: one index per partition); per 8-tile chunk one
     HWDGE store DMA writes the rows to the output.
"""

from contextlib import ExitStack

import numpy as np

import concourse.bass as bass
import concourse.bacc as bacc
import concourse.mybir as mybir
import concourse.tile as tile
from concourse import bass_utils
from concourse.masks import make_identity, make_upper_triangular

P = 128  # partitions = tokens per tile = G (blocks) = NG_MAX
KERNEL_VERSION = 7  # bump on every meaningful kernel change (NEFF-cache buster)
N_CORES = 8
B_FULL = 16  # full batch dim
N_TOK = 16384  # tokens per batch
D_FEAT = 256  # feature dim
NB = B_FULL // N_CORES  # batches per core

FP32 = mybir.dt.float32
BF16 = mybir.dt.bfloat16
I32 = mybir.dt.int32


def build_nc(NB: int, N: int, D: int, CT: int = 8, GRP: int = 4, LC: int = 16,
             STG_BUFS: int = 8, IDX_BUFS: int = 6, REPS: int = 1,
             DYN_LOOP: int = 0, MODE: str = "full", SCRATCH: int = 65536):
    """Build the per-core bass program.

    NB: batches per core; N: tokens per batch; D: feature dim.
    CT: tiles per store chunk; GRP: tiles per PSUM group; LC: tiles per
    onehot load chunk.
    """
    T = N // P  # token tiles per batch
    assert T * P == N
    GRP = min(GRP, T)
    CT = min(CT, T)
    LC = min(LC, T)
    assert T % GRP == 0 and T % CT == 0 and T % LC == 0
    add = mybir.AluOpType.add
    mult = mybir.AluOpType.mult
    bypass = mybir.AluOpType.bypass

    nc = bacc.Bacc("TRN2", target_bir_lowering=False, debug=False,
                   dynamic_dma_scratch_size=SCRATCH)

    feat = nc.dram_tensor("block_features", [NB * N, D], FP32, kind="ExternalInput")
    oh = nc.dram_tensor("block_onehot", [NB, N, P], FP32, kind="ExternalInput")
    out = nc.dram_tensor("out", [NB, N, D], FP32, kind="ExternalOutput")
    # The PJRT NEFF cache keys on the HLO alone (the embedded bass program
    # does not enter the hash), so distinct kernel versions collide.  A dummy
    # input whose shape encodes a version nonce forces a distinct hash.
    import zlib as _zlib
    _nonce = (
        _zlib.crc32(
            f"v{KERNEL_VERSION}-{NB}-{N}-{D}-{CT}-{GRP}-{LC}-{STG_BUFS}-{REPS}-{DYN_LOOP}-{MODE}-{SCRATCH}".encode()
        )
        % 4093
        + 1
    )
    nc.dram_tensor("version_tag", [1, _nonce], FP32, kind="ExternalInput")

    with tile.TileContext(nc) as tc, ExitStack() as ctx:
        cpool = ctx.enter_context(tc.tile_pool(name="const", bufs=1))
        ohpool = ctx.enter_context(tc.tile_pool(name="ohp", bufs=2))
        ldpool = ctx.enter_context(tc.tile_pool(name="ld", bufs=2))
        wpool = ctx.enter_context(tc.tile_pool(name="work", bufs=2))
        ppool = ctx.enter_context(tc.tile_pool(name="psum", bufs=2, space="PSUM"))
        pspool = ctx.enter_context(tc.tile_pool(name="psumsm", bufs=2, space="PSUM"))
        spool = ctx.enter_context(tc.tile_pool(name="stage", bufs=STG_BUFS))
        fpool = ctx.enter_context(tc.tile_pool(name="flat", bufs=2))

        # --- constants ---
        triu = cpool.tile([P, P], BF16)  # triu[k, m] = 1 iff k <= m
        make_upper_triangular(nc, triu[:], val=1.0, diag=True)
        ident = cpool.tile([P, P], FP32)
        make_identity(nc, ident[:])
        ones_col = cpool.tile([P, 1], BF16)
        nc.gpsimd.memset(ones_col[:], 1.0)
        ones_row = cpool.tile([1, P], FP32)
        nc.gpsimd.memset(ones_row[:], 1.0)
        # gmat[t, g] = 128*g - 1 (same every row)
        gmat_i = cpool.tile([P, P], I32)
        nc.gpsimd.iota(gmat_i[:], pattern=[[P, P]], base=-1, channel_multiplier=0)
        gmat = cpool.tile([P, P], FP32)
        nc.vector.tensor_copy(gmat[:], gmat_i[:])

        import contextlib
        loop_cm = tc.For_i(0, DYN_LOOP, 1) if DYN_LOOP else contextlib.nullcontext()
        with loop_cm:
          for rep in range(REPS):
            for b in range(NB):
                oh_sb = ohpool.tile([P, T * P], BF16, tag="oh")
                oh_src = oh.ap()[b].rearrange("(t p) g -> p t g", p=P)
                countsT_ps = pspool.tile([P, T], FP32, tag="counts")
                incl = wpool.tile([P, P], FP32, tag="incl")  # [g, t] inclusive
                pexT = wpool.tile([P, P], FP32, tag="pexT")  # [g, t] exclusive
                flat_mat = wpool.tile([P, P], FP32, tag="flatmat")  # [p', t]
                idx_mat = wpool.tile([P, P], I32, tag="idxmat")
                out_dst = out.ap()[b].rearrange("(t p) d -> p t d", p=P)

                if MODE == "gather":
                    idx_mat = wpool.tile([P, P], I32, tag="idxmat")
                    nc.gpsimd.iota(idx_mat[:, :T], pattern=[[1, T]], base=0,
                                   channel_multiplier=T)
                    for c in range(T // CT):
                        stg = spool.tile([P, CT * D], FP32, tag="stg")
                        for i in range(CT):
                            t = c * CT + i
                            nc.gpsimd.indirect_dma_start(
                                out=stg[:, i * D : (i + 1) * D],
                                out_offset=None,
                                in_=feat.ap(),
                                in_offset=bass.IndirectOffsetOnAxis(
                                    ap=idx_mat[:, t : t + 1], axis=0
                                ),
                                element_offset=b * N * D,
                            )
                        nc.sync.dma_start(
                            out=out_dst[:, c * CT : (c + 1) * CT, :], in_=stg[:]
                        )
                    continue
                PCH = min(16, T)  # tiles per prefix/transpose slice
                for s in range(T // PCH):
                    s0 = s * PCH
                    # --- load + cast + counts + scan per LC chunk ---
                    for lc in range(s0 // LC, (s0 + PCH) // LC):
                        lc0, lc1 = lc * LC, (lc + 1) * LC
                        ld = ldpool.tile([P, LC * P], FP32, tag="ld")
                        nc.scalar.dma_start(
                            out=ld[:], in_=oh_src[:, lc0:lc1, :]
                        )
                        nc.vector.tensor_copy(
                            oh_sb[:, lc0 * P : lc1 * P], ld[:]
                        )
                        for t in range(lc0, lc1):
                            nc.tensor.matmul(
                                out=countsT_ps[:, t : t + 1],
                                lhsT=oh_sb[:, t * P : (t + 1) * P],
                                rhs=ones_col[:],
                                start=True,
                                stop=True,
                            )
                        nc.vector.tensor_tensor_scan(
                            out=incl[:, lc0:lc1],
                            data0=countsT_ps[:, lc0:lc1],
                            data1=gmat[:, 0:LC],
                            initial=(0.0 if lc == 0 else incl[:, lc0 - 1 : lc0]),
                            op0=add,
                            op1=bypass,
                        )
                        nc.vector.tensor_tensor(
                            out=pexT[:, lc0:lc1],
                            in0=incl[:, lc0:lc1],
                            in1=countsT_ps[:, lc0:lc1],
                            op=mybir.AluOpType.subtract,
                        )
                    # --- transpose slice into partition-0 tiles + gmat add ---
                    pref_ps = pspool.tile([PCH, P], FP32, tag="preft")
                    nc.tensor.transpose(
                        out=pref_ps[:],
                        in_=pexT[:, s0 : s0 + PCH],
                        identity=ident[:],
                    )
                    pref_adj = wpool.tile([PCH, P], FP32, tag="prefadj")
                    nc.vector.tensor_tensor(
                        out=pref_adj[:],
                        in0=pref_ps[:],
                        in1=gmat[0:PCH, :],
                        op=add,
                    )
                    # --- groups: prefix broadcast + within-tile cumsum + select ---
                    for grp in range(s0 // GRP, (s0 + PCH) // GRP):
                        flat_row = fpool.tile([1, GRP * P], FP32, tag="flatrow")
                        g_in_s = grp - s0 // GRP
                        nc.scalar.dma_start(
                            out=flat_row[:],
                            in_=pref_adj[g_in_s * GRP : (g_in_s + 1) * GRP, :],
                        )
                        grp_ps = ppool.tile([P, GRP * P], FP32, tag="grp")
                        nc.tensor.matmul(
                            out=grp_ps[:],
                            lhsT=ones_row[:],
                            rhs=flat_row[:],
                            start=True,
                            stop=False,
                            skip_group_check=True,
                        )
                        for i in range(GRP):
                            t = grp * GRP + i
                            nc.tensor.matmul(
                                out=grp_ps[:, i * P : (i + 1) * P],
                                lhsT=triu[:],
                                rhs=oh_sb[:, t * P : (t + 1) * P],
                                start=False,
                                stop=True,
                                skip_group_check=True,
                            )
                        scratch = wpool.tile([P, GRP * P], FP32, tag="scratch")
                        for i in range(GRP):
                            t = grp * GRP + i
                            nc.vector.scalar_tensor_tensor(
                                out=scratch[:, i * P : (i + 1) * P],
                                in0=grp_ps[:, i * P : (i + 1) * P],
                                scalar=1.0,
                                in1=oh_sb[:, t * P : (t + 1) * P],
                                op0=mult,
                                op1=mult,
                                accum_out=flat_mat[:, t : t + 1],
                            )
                    if MODE == "index":
                        nc.sync.dma_start(
                            out=out_dst[:, s0 : s0 + 1, 0:1],
                            in_=flat_mat[:, s0 : s0 + 1],
                        )
                        continue
                    # --- gathers + stores for this slice ---
                    for c in range(s0 // CT, (s0 + PCH) // CT):
                        nc.vector.tensor_copy(
                            idx_mat[:, c * CT : (c + 1) * CT],
                            flat_mat[:, c * CT : (c + 1) * CT],
                        )
                        stg = spool.tile([P, CT * D], FP32, tag="stg")
                        for i in range(CT):
                            t = c * CT + i
                            nc.gpsimd.indirect_dma_start(
                                out=stg[:, i * D : (i + 1) * D],
                                out_offset=None,
                                in_=feat.ap(),
                                in_offset=bass.IndirectOffsetOnAxis(
                                    ap=idx_mat[:, t : t + 1], axis=0
                                ),
                                element_offset=b * N * D,
                            )
                        nc.sync.dma_start(
                            out=out_dst[:, c * CT : (c + 1) * CT, :], in_=stg[:]
                        )

    nc.compile()
    return nc


_NC_CACHE = {}


def _get_nc():
    key = (NB, N_TOK, D_FEAT)
    if key not in _NC_CACHE:
        _NC_CACHE[key] = build_nc(*key)
    return _NC_CACHE[key]


def make_in_maps(block_features: np.ndarray, block_onehot: np.ndarray):
    """Shard full inputs batch-wise into 8 per-core input maps."""
    feat = np.ascontiguousarray(block_features, dtype=np.float32).reshape(
        B_FULL, N_TOK, D_FEAT
    )
    oh = np.ascontiguousarray(block_onehot, dtype=np.float32)
    nc = _get_nc()
    tag_shape = None
    for alloc in nc.m.functions[0].allocations:
        if isinstance(alloc, mybir.MemoryLocationSet) and alloc.kind == "ExternalInput":
            if alloc.memorylocations[0].name == "version_tag":
                tag_shape = tuple(alloc.tensor_shape)
    in_maps = []
    for c in range(N_CORES):
        lo, hi = c * NB, (c + 1) * NB
        m = {
            "block_features": feat[lo:hi].reshape(NB * N_TOK, D_FEAT),
            "block_onehot": oh[lo:hi],
        }
        if tag_shape is not None:
            m["version_tag"] = np.zeros(tag_shape, np.float32)
        in_maps.append(m)
    return in_maps


def run_spmd(in_maps, trace: bool = False):
    """Compile (cached) + run the SPMD program on cores 0-7."""
    nc = _get_nc()
    return bass_utils.run_bass_kernel_spmd(
        nc, in_maps, core_ids=list(range(N_CORES)), trace=trace
    )


def kernel(**inputs) -> np.ndarray:
    block_features = inputs["block_features"]
    block_onehot = inputs["block_onehot"]
    in_maps = make_in_maps(block_features, block_onehot)
    res = run_spmd(in_maps, trace=False)
    out = np.concatenate([r["out"] for r in res.results], axis=0)
    return out.reshape(B_FULL, N_TOK, D_FEAT)



# revision 9
# speedup vs baseline: 6.0714x; 6.0714x over previous
"""Trainium2 Bass kernel for nn_BlockUngrouper.

Problem: out[b, n, :] = block_features[b, g, k, :] where g is the block whose
one-hot claims token n and k is n's rank within that block (cumsum of the
one-hot along n).  The input distribution (per-sample permutation partition)
guarantees each token is claimed by exactly one block and ranks < 128, so per
batch this is a row-permutation gather with
    flat_idx[n] = 128 * g(n) + rank(n).

Sharding: data-parallel over the batch dim, 2 batches per NeuronCore x 8.

Per-core program, per batch, per slice of 16 token-tiles (2048 tokens):
  1. onehot slice -> SBUF bf16 via casting SWDGE DMA (fp32->bf16 in flight).
  2. counts[g, i]: per 128-token tile, PE matmul lhsT=OH rhs=ones.
  3. incl/pexT[g, t]: running scan over tiles (DVE tensor_tensor_scan),
     PE-transpose the slice to [t, g], add 128*g - 1.
  4. per 4-tile PSUM group: broadcast each tile's prefix row via a K=1
     matmul reading the single-partition row of pref_adj directly, then a
     triangular matmul adds the within-tile inclusive cumsum.
  5. flat_mat[p, r]: DVE scalar_tensor_tensor (PSUM x onehot) with accum_out
     reducing over g (the one-hot selects its block's entry).
  6. fold flat_mat [128, 16] into the dma_gather index layout [16, 128]
     int16 (idx for logical token i at [i%16, i//16]): 8 identity-slice
     matmuls move partitions 16v+q -> q, 8 strided DVE copies cast+place.
  7. one SWDGE dma_gather fetches all 2048 feature rows (1 KB each) of the
     slice into SBUF [128, 16, 1024B]; one HWDGE store DMA writes them out.

vs the v7 baseline this removes the per-tile indirect DMAs whose 994 ns
fixed SWDGE overhead made Pool the bottleneck (91% busy): descriptor
generation drops from 256 x 1038 ns to 16 x ~1.7 us per core.
"""

from contextlib import ExitStack

import numpy as np

import concourse.bass as bass
import concourse.bacc as bacc
import concourse.mybir as mybir
import concourse.tile as tile
from concourse import bass_utils
from concourse.masks import make_identity, make_upper_triangular

P = 128  # partitions = tokens per tile = G (blocks) = NG_MAX
KERNEL_VERSION = 8  # bump on every meaningful kernel change (NEFF-cache buster)
N_CORES = 8
B_FULL = 16  # full batch dim
N_TOK = 16384  # tokens per batch
D_FEAT = 256  # feature dim
NB = B_FULL // N_CORES  # batches per core

FP32 = mybir.dt.float32
BF16 = mybir.dt.bfloat16
I16 = mybir.dt.int16
I32 = mybir.dt.int32


def build_nc(NB: int, N: int, D: int, SLICE: int = 16, GRP: int = 4,
             STG_BUFS: int = 3, OH_BUFS: int = 3, REPS: int = 1,
             DYN_LOOP: int = 0, SCRATCH: int = 131072, NQ: int = 1,
             GQ: int = 0, SP: int = 0, CASTLOAD: int = 1, ANTGATHER: int = 1):
    """Build the per-core bass program.

    NB: batches per core; N: tokens per batch; D: feature dim.
    SLICE: token-tiles per slice (= gather-chunk = store-chunk);
    GRP: tiles per PSUM group.
    """
    T = N // P  # token tiles per batch
    assert T * P == N
    SLICE = min(SLICE, T)
    assert T % SLICE == 0 and SLICE % GRP == 0 and SLICE % 8 == 0
    add = mybir.AluOpType.add
    mult = mybir.AluOpType.mult
    bypass = mybir.AluOpType.bypass

    nc = bacc.Bacc("TRN2", target_bir_lowering=False, debug=False,
                   dynamic_dma_scratch_size=SCRATCH, num_swdge_queues=NQ)

    feat = nc.dram_tensor("block_features", [NB * N, D], FP32, kind="ExternalInput")
    oh = nc.dram_tensor("block_onehot", [NB, N, P], FP32, kind="ExternalInput")
    out = nc.dram_tensor("out", [NB, N, D], FP32, kind="ExternalOutput")
    # The PJRT NEFF cache keys on the HLO alone (the embedded bass program
    # does not enter the hash), so distinct kernel versions collide.  A dummy
    # input whose shape encodes a version nonce forces a distinct hash.
    import zlib as _zlib
    _nonce = (
        _zlib.crc32(
            f"v{KERNEL_VERSION}-{NB}-{N}-{D}-{SLICE}-{GRP}-{STG_BUFS}-{OH_BUFS}-{REPS}-{DYN_LOOP}-{SCRATCH}-{NQ}-{GQ}-{SP}-{CASTLOAD}-{ANTGATHER}".encode()
        )
        % 4093
        + 1
    )
    nc.dram_tensor("version_tag", [1, _nonce], FP32, kind="ExternalInput")

    with tile.TileContext(nc) as tc, ExitStack() as ctx:
        cpool = ctx.enter_context(tc.tile_pool(name="const", bufs=1))
        ohpool = ctx.enter_context(tc.tile_pool(name="ohp", bufs=OH_BUFS))
        wpool = ctx.enter_context(tc.tile_pool(name="work", bufs=2))
        ppool = ctx.enter_context(tc.tile_pool(name="psum", bufs=2, space="PSUM"))
        pspool = ctx.enter_context(tc.tile_pool(name="psumsm", bufs=2, space="PSUM"))
        ipool = ctx.enter_context(tc.tile_pool(name="idxp", bufs=3))
        spool = ctx.enter_context(tc.tile_pool(name="stage", bufs=STG_BUFS))

        # --- constants ---
        triu = cpool.tile([P, P], BF16)  # triu[k, m] = 1 iff k <= m
        make_upper_triangular(nc, triu[:], val=1.0, diag=True)
        ident = cpool.tile([P, P], FP32)
        make_identity(nc, ident[:])
        ones_col = cpool.tile([P, 1], BF16)
        nc.gpsimd.memset(ones_col[:], 1.0)
        ones_row = cpool.tile([1, P], FP32)
        nc.gpsimd.memset(ones_row[:], 1.0)
        # gmat[t, g] = 128*g - 1 (same every row)
        gmat_i = cpool.tile([P, P], I32)
        nc.gpsimd.iota(gmat_i[:], pattern=[[P, P]], base=-1, channel_multiplier=0)
        gmat = cpool.tile([P, P], FP32)
        nc.vector.tensor_copy(gmat[:], gmat_i[:])
        # brep[k, 16u+w] = 1 iff w == k: broadcast 16 partitions -> 128 with
        # period 16 (the dma_gather idx stripes for the 8 GPSIMD cores)
        brep = cpool.tile([16, P], FP32)
        nc.gpsimd.memset(brep[:], 0.0)
        nc.gpsimd.affine_select(
            out=brep[:],
            in_=brep[:],
            compare_op=mybir.AluOpType.not_equal,
            fill=1.0,
            base=0,
            pattern=[[0, P // 16], [-1, 16]],
            channel_multiplier=1,
        )

        import contextlib
        loop_cm = tc.For_i(0, DYN_LOOP, 1) if DYN_LOOP else contextlib.nullcontext()
        with loop_cm:
          for rep in range(REPS):
            for b in range(NB):
                oh_src = oh.ap()[b].rearrange("(t p) g -> p t g", p=P)
                out_dst = out.ap()[b].rearrange("(t p) d -> p t d", p=P)
                feat_b = feat.ap()[b * N : (b + 1) * N, :]
                incl = wpool.tile([P, P], FP32, tag="incl")  # [g, t] inclusive
                pexT = wpool.tile([P, P], FP32, tag="pexT")  # [g, t] exclusive

                for s in range(T // SLICE):
                    s0 = s * SLICE
                    # --- load + cast (SWDGE queue 0) ---
                    oh_sl = ohpool.tile([P, SLICE * P], BF16, tag="oh")
                    if CASTLOAD:
                        nc.gpsimd.dma_start(
                            out=oh_sl[:].rearrange("p (t g) -> p t g", g=P),
                            in_=oh_src[:, s0 : s0 + SLICE, :],
                        )
                    else:
                        ld = ohpool.tile([P, SLICE * P], FP32, tag="ld")
                        nc.scalar.dma_start(
                            out=ld[:].rearrange("p (t g) -> p t g", g=P),
                            in_=oh_src[:, s0 : s0 + SLICE, :],
                        )
                        nc.scalar.tensor_copy(oh_sl[:], ld[:])
                    # --- per-tile counts: [g, i] for slice tiles ---
                    # one PSUM bank shared by the slice's small tensors:
                    # cols 0:128 pref_ps, 128:256 fold_ps, 256:384 rep_ps,
                    # 384:384+SLICE countsT
                    small = pspool.tile([P, 512], FP32, tag="small")
                    countsT = small[:, 384 : 384 + SLICE]
                    for i in range(SLICE):
                        nc.tensor.matmul(
                            out=countsT[:, i : i + 1],
                            lhsT=oh_sl[:, i * P : (i + 1) * P],
                            rhs=ones_col[:],
                            start=True,
                            stop=True,
                            skip_group_check=True,
                        )
                    # --- running scan over tiles, exclusive prefix ---
                    nc.vector.tensor_tensor_scan(
                        out=incl[:, s0 : s0 + SLICE],
                        data0=countsT,
                        data1=gmat[:, 0:SLICE],
                        initial=(0.0 if s == 0 else incl[:, s0 - 1 : s0]),
                        op0=add,
                        op1=bypass,
                    )
                    nc.vector.tensor_tensor(
                        out=pexT[:, s0 : s0 + SLICE],
                        in0=incl[:, s0 : s0 + SLICE],
                        in1=countsT,
                        op=mybir.AluOpType.subtract,
                    )
                    # --- transpose slice to [t, g], add 128*g - 1 ---
                    pref_ps = small[0:SLICE, 0:P]
                    nc.tensor.matmul(
                        out=pref_ps,
                        lhsT=pexT[:, s0 : s0 + SLICE],
                        rhs=ident[:],
                        is_transpose=True,
                        skip_group_check=True,
                    )
                    pref_adj = wpool.tile([SLICE, P], FP32, tag="prefadj")
                    nc.vector.tensor_tensor(
                        out=pref_adj[:],
                        in0=pref_ps,
                        in1=gmat[0:SLICE, :],
                        op=add,
                    )
                    # flatten the slice's prefix rows onto partition 0 (the
                    # PE requires rhs base partition 0 for the K=1 broadcast)
                    flat_row = wpool.tile([1, SLICE * P], FP32, tag="flatrow")
                    nc.scalar.dma_start(out=flat_row[:], in_=pref_adj[:])
                    # --- groups: prefix broadcast + within-tile cumsum + select ---
                    flat_mat = wpool.tile([P, SLICE], FP32, tag="flat")
                    for g in range(SLICE // GRP):
                        grp_ps = ppool.tile([P, GRP * P], FP32, tag="grp")
                        scratch = wpool.tile([P, GRP * P], FP32, tag="scratch")
                        for i in range(GRP):
                            r = g * GRP + i  # tile within slice
                            nc.tensor.matmul(
                                out=grp_ps[:, i * P : (i + 1) * P],
                                lhsT=ones_row[:],
                                rhs=flat_row[:, r * P : (r + 1) * P],
                                start=True,
                                stop=False,
                                skip_group_check=True,
                            )
                            nc.tensor.matmul(
                                out=grp_ps[:, i * P : (i + 1) * P],
                                lhsT=triu[:],
                                rhs=oh_sl[:, r * P : (r + 1) * P],
                                start=False,
                                stop=True,
                                skip_group_check=True,
                            )
                        for i in range(GRP):
                            r = g * GRP + i
                            nc.vector.scalar_tensor_tensor(
                                out=scratch[:, i * P : (i + 1) * P],
                                in0=grp_ps[:, i * P : (i + 1) * P],
                                scalar=1.0,
                                in1=oh_sl[:, r * P : (r + 1) * P],
                                op0=mult,
                                op1=mult,
                                accum_out=flat_mat[:, r : r + 1],
                            )
                    # --- fold to dma_gather idx layout [128, SLICE*8] i16 ---
                    # idx of logical token i=r*128+p sits at [i%16, i//16] =
                    # [p%16, r*8 + p//16]; fold_v[q, r] = flat_mat[16v+q, r]
                    # lands at column r*8+v; the 16-partition wrap is then
                    # replicated to all 128 partitions (8 GPSIMD core stripes)
                    # via the brep matmul.
                    fold_ps = small[0:16, P : P + 8 * SLICE]
                    for v in range(8):
                        nc.tensor.matmul(
                            out=fold_ps[:, v * SLICE : (v + 1) * SLICE],
                            lhsT=ident[:, 16 * v : 16 * (v + 1)],
                            rhs=flat_mat[:],
                            start=True,
                            stop=True,
                            skip_group_check=True,
                        )
                    fold_sb = wpool.tile([16, SLICE * 8], FP32, tag="foldsb")
                    fold_dst = fold_sb[:].rearrange("q (r v) -> q r v", v=8)
                    fold_src = fold_ps.rearrange("q (v r) -> q v r", v=8)
                    for v in range(8):
                        nc.vector.tensor_copy(
                            fold_dst[:, :, v], fold_src[:, v, :]
                        )
                    rep_ps = small[:, 2 * P : 2 * P + SLICE * 8]
                    nc.tensor.matmul(
                        out=rep_ps,
                        lhsT=brep[:],
                        rhs=fold_sb[:],
                        start=True,
                        stop=True,
                        skip_group_check=True,
                    )
                    idxs16 = ipool.tile([P, SLICE * 8], I16, tag="idx")
                    nc.vector.tensor_copy(idxs16[:], rep_ps)
                    # --- gather (SWDGE queue GQ) + store ---
                    stg = spool.tile([P, SLICE * D], FP32, tag="stg")
                    if ANTGATHER:
                        nc.gpsimd.dma_gather(
                            out_ap=stg[:].rearrange("p (c d) -> p c d", d=D),
                            in_ap=feat_b,
                            idxs_ap=idxs16[:],
                            num_idxs=SLICE * P,
                            num_idxs_reg=SLICE * P,
                            elem_size=D,
                            queue_num=GQ,
                            single_packet=bool(SP),
                        )
                    else:
                        idx32 = ipool.tile([P, SLICE], I32, tag="idx32")
                        nc.vector.tensor_copy(idx32[:], flat_mat[:])
                        for i in range(SLICE):
                            nc.gpsimd.indirect_dma_start(
                                out=stg[:, i * D : (i + 1) * D],
                                out_offset=None,
                                in_=feat.ap(),
                                in_offset=bass.IndirectOffsetOnAxis(
                                    ap=idx32[:, i : i + 1], axis=0
                                ),
                                element_offset=b * N * D,
                            )
                    nc.sync.dma_start(
                        out=out_dst[:, s0 : s0 + SLICE, :],
                        in_=stg[:].rearrange("p (c d) -> p c d", d=D),
                    )

    nc.compile()
    return nc


_NC_CACHE = {}


def _get_nc():
    key = (NB, N_TOK, D_FEAT)
    if key not in _NC_CACHE:
        _NC_CACHE[key] = build_nc(*key)
    return _NC_CACHE[key]


def make_in_maps(block_features: np.ndarray, block_onehot: np.ndarray):
    """Shard full inputs batch-wise into 8 per-core input maps."""
    feat = np.ascontiguousarray(block_features, dtype=np.float32).reshape(
        B_FULL, N_TOK, D_FEAT
    )
    oh = np.ascontiguousarray(block_onehot, dtype=np.float32)
    nc = _get_nc()
    tag_shape = None
    for alloc in nc.m.functions[0].allocations:
        if isinstance(alloc, mybir.MemoryLocationSet) and alloc.kind == "ExternalInput":
            if alloc.memorylocations[0].name == "version_tag":
                tag_shape = tuple(alloc.tensor_shape)
    in_maps = []
    for c in range(N_CORES):
        lo, hi = c * NB, (c + 1) * NB
        m = {
            "block_features": feat[lo:hi].reshape(NB * N_TOK, D_FEAT),
            "block_onehot": oh[lo:hi],
        }
        if tag_shape is not None:
            m["version_tag"] = np.zeros(tag_shape, np.float32)
        in_maps.append(m)
    return in_maps


def run_spmd(in_maps, trace: bool = False):
    """Compile (cached) + run the SPMD program on cores 0-7."""
    nc = _get_nc()
    return bass_utils.run_bass_kernel_spmd(
        nc, in_maps, core_ids=list(range(N_CORES)), trace=trace
    )


def kernel(**inputs) -> np.ndarray:
    block_features = inputs["block_features"]
    block_onehot = inputs["block_onehot"]
    in_maps = make_in_maps(block_features, block_onehot)
    res = run_spmd(in_maps, trace=False)
    out = np.concatenate([r["out"] for r in res.results], axis=0)
    return out.reshape(B_FULL, N_TOK, D_FEAT)
